# revision 10
# baseline (speedup 1.0000x reference)
"""Fused LoRA-attention block (qkv + k/v LoRA + MHA softmax + out-proj) for
Trainium2, data-parallel over batch across 8 NeuronCores.

Per-core layout strategy (batch shard = 2 of 16):
  - Host folds the rank-64 LoRA into W_k/W_v (W + (alpha/r) B@A, fp32), folds
    the V bias and proj bias into one output bias (softmax rows sum to 1), and
    drops the K bias entirely (softmax is invariant to the per-query constant
    q.bk). Everything is pre-transposed so each matmul's contraction lands on
    SBUF partitions; matmul data bf16 (fp32 PSUM), softmax statistics fp32.
  - Q^T/K^T computed channel-major [c_out, tok]; V token-major [tok, c_out]
    with an appended ones column per head so the attention row-sum falls out
    of the P@V matmul for free (row 64 of the [65, q] PSUM tile).
  - S^T = K@Q^T per head pair with k on partitions; the two heads of a pair
    run concurrently in separate PE row groups (K=64 each) and land in one
    [128, 1024] two-bank PSUM tile, so a single ACT exp instruction covers
    both heads (softmax runs without max-subtraction, logits bounded ~|3|).
  - The attention stream is ACT-throughput-limited (exp), so the surrounding
    gemm matmuls are interleaved into the attention emission at k-block
    granularity — the in-order PE queue then always has ready work while exp
    for the next AV completes. Each batch's slots carry the PREVIOUS batch's
    out-projection (oT is double-buffered by batch parity) followed by the
    NEXT batch's qkv; qkv chains for channel block j are released only after
    attention pair j (qT/kT WAR), i.e. one slot late.
  - Both streams software-pipeline across For_i reps (first qkv runs once
    before the loop; the last batch's projection is an epilogue after it).
  - PSUM: 4 banks S (double-buffered), 2 banks AV ([65,1024], both heads),
    2 banks gemm chains. Paired N=512 chains share each stationary load by
    interleaving the two token-halves of a PSUM pair.
"""

import sys

sys.path.insert(0, "/opt/trn_rl_repo")

import ml_dtypes
import numpy as np

import concourse.bass as bass
import concourse.mybir as mybir
import concourse.tile as tile
from concourse import bacc
from concourse.bass_utils import run_bass_kernel_spmd

NCORES = 8
B, N, C = 16, 1024, 1024
H, D, R = 16, 64, 64
BSH = B // NCORES  # batches per core
NB = C // 128  # channel blocks
SCALE = D**-0.5
LSCALE = 1.0 / R
BF = mybir.dt.bfloat16
F32 = mybir.dt.float32
BF_NP = ml_dtypes.bfloat16
HALVES = (bass.ts(0, 512), bass.ts(1, 512))
UNIT_EMISSIONS = 3 * (2 * NB + 2)  # Q,K,V chains: 16 matmuls + 2 drains each
PROJ_EMISSIONS = NB * (2 * NB + 4)  # 8 chains: 16 matmuls + 2 drains + dma + pad


def build_nc(
    loop_reps: int = 1,
    interleave: bool = True,
    proj_pipe: bool = True,
    probe_noexp: bool = False,
    probe_nonorm: bool = False,
):
    nc = bacc.Bacc(None, target_bir_lowering=False, debug=False)

    xt_d = nc.dram_tensor("xt", [BSH, NB, 128, N], BF, kind="ExternalInput")
    wq_d = nc.dram_tensor("wq", [NB, 128, C], BF, kind="ExternalInput")
    wk_d = nc.dram_tensor("wk", [NB, 128, C], BF, kind="ExternalInput")
    wv_d = nc.dram_tensor("wv", [NB, 128, C], BF, kind="ExternalInput")
    wp_d = nc.dram_tensor("wp", [NB, 128, C], BF, kind="ExternalInput")
    bq_d = nc.dram_tensor("bq", [128, NB], F32, kind="ExternalInput")
    bo_d = nc.dram_tensor("bo", [128, C], BF, kind="ExternalInput")
    out_d = nc.dram_tensor("out", [BSH, N, C], BF, kind="ExternalOutput")

    with tile.TileContext(nc) as tc:
        with (
            tc.tile_pool(name="wpool", bufs=1) as wpool,
            tc.tile_pool(name="xtp", bufs=1) as xtp,
            tc.tile_pool(name="actp", bufs=1) as actp,
            tc.tile_pool(name="ptp", bufs=3) as ptp,
            tc.tile_pool(name="rsp", bufs=1) as rsp,
            tc.tile_pool(name="outp", bufs=1) as outp,
            tc.tile_pool(name="gmps", bufs=2, space="PSUM") as gmps,
            tc.tile_pool(name="sps_p", bufs=2, space="PSUM") as sps_p,
            tc.tile_pool(name="avps", bufs=1, space="PSUM") as avps,
        ):
            # ---- persistent weights ----
            wq_sb = wpool.tile([128, NB, C], BF, tag="wq")
            wk_sb = wpool.tile([128, NB, C], BF, tag="wk")
            wv_sb = wpool.tile([128, NB, C], BF, tag="wv")
            wp_sb = wpool.tile([128, NB, C], BF, tag="wp")
            for w_sb, w_d in ((wq_sb, wq_d), (wk_sb, wk_d), (wv_sb, wv_d), (wp_sb, wp_d)):
                for blk in range(NB):
                    nc.sync.dma_start(out=w_sb[:, blk, :], in_=w_d[blk])
            bq_sb = wpool.tile([128, NB], F32, tag="bq")
            nc.sync.dma_start(out=bq_sb[:], in_=bq_d[:])
            bo_sb = wpool.tile([128, C], BF, tag="bo")
            nc.sync.dma_start(out=bo_sb[:], in_=bo_d[:])

            # V with per-head ones column appended: [128, tblk, head, 65]
            vaug0 = wpool.tile([128, NB, H, D + 1], BF, tag="vaug0")
            vaug1 = wpool.tile([128, NB, H, D + 1], BF, tag="vaug1")
            vaugs = (vaug0, vaug1)
            nc.vector.memset(vaug0[:, :, :, D : D + 1], 1.0)
            nc.vector.memset(vaug1[:, :, :, D : D + 1], 1.0)
            if probe_noexp:
                # timing probe: AV consumes this constant tile instead of the
                # exp output, removing ACT work and the S->exp->AV dependency.
                pt_const = wpool.tile([128, 1024], BF, tag="pt_const")
                nc.vector.memset(pt_const[:], 0.001)
            qT_blk = [
                actp.tile([128, N], BF, tag=f"qT{cb}", name=f"qT{cb}")
                for cb in range(NB)
            ]
            kT_blk = [
                actp.tile([128, N], BF, tag=f"kT{cb}", name=f"kT{cb}")
                for cb in range(NB)
            ]
            # oT double-buffered by batch parity so the previous batch's
            # projection overlaps this batch's attention.
            nbuf_oT = 2 if proj_pipe else 1
            oT_blk = [
                [
                    actp.tile([128, N], BF, tag=f"oT{p}_{cb}", name=f"oT{p}_{cb}")
                    for cb in range(NB)
                ]
                for p in range(nbuf_oT)
            ]

            def gemm_chain(emit_mm, drain, n_steps):
                """Generator: paired 512-wide PSUM chains, yield per emission."""
                pss = [
                    gmps.tile([128, 512], F32, tag="gm", name=f"gm{i}")
                    for i in range(2)
                ]
                for step in range(n_steps):
                    for i, hv in enumerate(HALVES):
                        emit_mm(pss[i], hv, step)
                        yield
                for i, hv in enumerate(HALVES):
                    drain(pss[i], hv, i)
                    yield

            def q_chain(xt_sb, cb):
                csl = bass.ts(cb, 128)

                def mm(ps, hv, ci):
                    nc.tensor.matmul(
                        ps[:],
                        wq_sb[:, ci, csl],
                        xt_sb[:, ci, hv],
                        start=(ci == 0),
                        stop=(ci == NB - 1),
                    )

                def drain(ps, hv, i):
                    nc.vector.tensor_scalar_add(
                        qT_blk[cb][:, hv], ps[:], bq_sb[:, cb : cb + 1]
                    )

                return gemm_chain(mm, drain, NB)

            def k_chain(xt_sb, cb):
                csl = bass.ts(cb, 128)

                def mm(ps, hv, ci):
                    nc.tensor.matmul(
                        ps[:],
                        wk_sb[:, ci, csl],
                        xt_sb[:, ci, hv],
                        start=(ci == 0),
                        stop=(ci == NB - 1),
                    )

                def drain(ps, hv, i):
                    nc.vector.tensor_copy(kT_blk[cb][:, hv], ps[:])

                return gemm_chain(mm, drain, NB)

            def v_chain(b, xt_sb, tb):
                vaug_sb = vaugs[b % 2]
                tsl = bass.ts(tb, 128)

                def mm(ps, hv, step):
                    nc.tensor.matmul(
                        ps[:],
                        xt_sb[:, step, tsl],
                        wv_sb[:, step, hv],
                        start=(step == 0),
                        stop=(step == NB - 1),
                    )

                def drain(ps, hv, i):
                    nc.vector.tensor_copy(
                        vaug_sb[:, tb, i * 8 : (i + 1) * 8, 0:D],
                        ps[:].rearrange("p (h d) -> p h d", d=D),
                    )

                return gemm_chain(mm, drain, NB)

            def proj_chain(b, qb):
                qsl = bass.ts(qb, 128)
                oT = oT_blk[b % nbuf_oT]

                def mm_p(ps, hv, step):
                    nc.tensor.matmul(
                        ps[:],
                        oT[step][:, qsl],
                        wp_sb[:, step, hv],
                        start=(step == 0),
                        stop=(step == NB - 1),
                    )

                pss = [
                    gmps.tile([128, 512], F32, tag="gm", name=f"gm{i}")
                    for i in range(2)
                ]
                for step in range(NB):
                    for i, hv in enumerate(HALVES):
                        mm_p(pss[i], hv, step)
                        yield
                ost = outp.tile([128, N], BF, tag="ost")
                for i, hv in enumerate(HALVES):
                    nc.vector.tensor_add(ost[:, hv], pss[i][:], bo_sb[:, hv])
                    yield
                nc.sync.dma_start(out=out_d[b, qsl, :], in_=ost[:])
                yield

            def proj_stream(b):
                for qb in range(NB):
                    yield from proj_chain(b, qb)

            def gemm_stream(b, xt_sb):
                for j in range(NB):
                    yield from q_chain(xt_sb, j)
                    yield from k_chain(xt_sb, j)
                    yield from v_chain(b, xt_sb, j)

            class Puller:
                def __init__(self, stream):
                    self.stream = stream
                    self.pulled = 0
                    self.done = stream is None

                def pull(self, n, cap):
                    while n > 0 and not self.done and self.pulled < cap:
                        try:
                            next(self.stream)
                            self.pulled += 1
                        except StopIteration:
                            self.done = True
                        n -= 1

                def drain_all(self):
                    while not self.done:
                        try:
                            next(self.stream)
                        except StopIteration:
                            self.done = True

            def emit_xt(b):
                xt_sb = xtp.tile([128, NB, N], BF, tag="xt", name="xt")
                for blk in range(NB):
                    nc.sync.dma_start(out=xt_sb[:, blk, :], in_=xt_d[b, blk])
                return xt_sb

            def attention_pair(b, pr, puller, base_cap):
                # qkv chains for channel block j only after attention pair j:
                # during slot pr, units 0..pr-1 are eligible (plus base_cap
                # unconstrained proj emissions at the stream head).
                cap = base_cap + pr * UNIT_EMISSIONS
                vaug_sb = vaugs[b % 2]
                offs = (0, 64)
                for hv_i, hv in enumerate(HALVES):
                    # [65, 1024]: head a in [:, 0:512], head b in [:, 512:1024];
                    # row 64 is the softmax denominator.
                    avp_t = avps.tile([D + 1, 1024], F32, tag="avp", name="avp")
                    avs = [avp_t[:, bass.ts(0, 512)], avp_t[:, bass.ts(1, 512)]]

                    def emit_av(pts, kb_):
                        for hi in range(2):
                            h = 2 * pr + hi
                            nc.tensor.matmul(
                                avs[hi],
                                vaug_sb[:, kb_, h, :],
                                pts[hi],
                                start=(kb_ == 0),
                                stop=(kb_ == NB - 1),
                            )

                    pend = None
                    for kb_ in range(NB):
                        ksl = bass.ts(kb_, 128)
                        sp = sps_p.tile([128, 1024], F32, tag="sp", name="sp")
                        for hi, off in enumerate(offs):
                            nc.tensor.matmul(
                                sp[:, bass.ts(hi, 512)],
                                kT_blk[pr][off : off + D, ksl],
                                qT_blk[pr][off : off + D, hv],
                                start=True,
                                stop=True,
                            )
                        if probe_noexp:
                            pt = pt_const
                        else:
                            pt = ptp.tile([128, 1024], BF, tag="pT", name="pT")
                            nc.scalar.activation(
                                pt[:], sp[:], mybir.ActivationFunctionType.Exp
                            )
                        pts = [pt[:, bass.ts(0, 512)], pt[:, bass.ts(1, 512)]]
                        puller.pull(3, cap)
                        if pend is not None:
                            emit_av(pend[0], pend[1])
                        pend = (pts, kb_)
                        puller.pull(3, cap)
                    emit_av(pend[0], pend[1])

                    if probe_nonorm:
                        for hi, off in enumerate(offs):
                            nc.vector.tensor_copy(
                                oT_blk[b % nbuf_oT][pr][off : off + D, hv],
                                avs[hi][0:D],
                            )
                        continue
                    # one copy releases the AV PSUM tile for the next half's
                    # accumulation; the norm math runs off the SBUF copy.
                    cav = rsp.tile([D + 1, 1024], F32, tag="cav")
                    nc.vector.tensor_copy(cav[:], avp_t[:])
                    rs = rsp.tile([1, 1024], F32, tag="rs")
                    nc.vector.reciprocal_approx_fast(rs[:], cav[D : D + 1, :])
                    bc = rsp.tile([D, 1024], F32, tag="bc")
                    nc.gpsimd.partition_broadcast(bc[:], rs[:])
                    for hi, off in enumerate(offs):
                        nc.vector.tensor_mul(
                            oT_blk[b % nbuf_oT][pr][off : off + D, hv],
                            cav[0:D, bass.ts(hi, 512)],
                            bc[:, bass.ts(hi, 512)],
                        )

            def body():
                for b in range(BSH):
                    nxt = (b + 1) % BSH
                    prv = (b - 1) % BSH
                    xt_next = emit_xt(nxt)

                    def full_stream():
                        if proj_pipe:
                            yield from proj_stream(prv)
                        yield from gemm_stream(nxt, xt_next)

                    base_cap = PROJ_EMISSIONS if proj_pipe else 0
                    puller = Puller(full_stream() if interleave else None)
                    for pr in range(H // 2):
                        attention_pair(b, pr, puller, base_cap)
                        if not interleave:
                            if pr < NB:
                                for _ in q_chain(xt_next, pr):
                                    pass
                                for _ in k_chain(xt_next, pr):
                                    pass
                                for _ in v_chain(nxt, xt_next, pr):
                                    pass
                    puller.drain_all()
                    if not proj_pipe:
                        p = Puller(proj_stream(b))
                        p.drain_all()

            # prologue: first batch's qkv once; zero oT[1] so the first
            # (discarded, later overwritten) pipelined projection has inputs.
            xt0 = emit_xt(0)
            p0 = Puller(gemm_stream(0, xt0))
            p0.drain_all()
            if proj_pipe:
                for cb in range(NB):
                    nc.vector.memset(oT_blk[1][cb][:], 0.0)

            if loop_reps > 1:
                with tc.For_i(0, loop_reps, 1):
                    body()
            else:
                body()

            if proj_pipe:
                pe = Puller(proj_stream(BSH - 1))
                pe.drain_all()

    nc.compile()
    return nc


def _prep_shared(W_qkv, b_qkv, lora_kA, lora_kB, lora_vA, lora_vB, W_proj, b_proj):
    def bf(a):
        return np.ascontiguousarray(a).astype(BF_NP)

    W_qkv = np.asarray(W_qkv, np.float32)
    W_proj = np.asarray(W_proj, np.float32)
    lora_kA = np.asarray(lora_kA, np.float32)
    lora_kB = np.asarray(lora_kB, np.float32)
    lora_vA = np.asarray(lora_vA, np.float32)
    lora_vB = np.asarray(lora_vB, np.float32)
    b_qkv = np.asarray(b_qkv, np.float32)
    b_proj = np.asarray(b_proj, np.float32)

    # Fold LoRA into the k/v weights (fp32 on host).
    Wk_eff = W_qkv[C : 2 * C] + LSCALE * (lora_kB @ lora_kA)
    Wv_eff = W_qkv[2 * C :] + LSCALE * (lora_vB @ lora_vA)
    # Softmax rows sum to 1, so the V bias rides through attention unchanged:
    # out = attn@(xWv^T)@Wp^T + (Wp bv + bp). The K bias only adds a
    # per-query constant to the logits, which softmax ignores — dropped.
    bv = b_qkv[2 * C :]
    bo = b_proj + W_proj @ bv
    return {
        "wq": bf((W_qkv[:C].T * SCALE).reshape(NB, 128, C)),
        "wk": bf(Wk_eff.T.reshape(NB, 128, C)),
        "wv": bf(Wv_eff.T.reshape(NB, 128, C)),
        "wp": bf(W_proj.T.reshape(NB, 128, C)),
        "bq": np.ascontiguousarray((b_qkv[:C] * SCALE).reshape(NB, 128).T),
        "bo": bf(np.broadcast_to(bo.reshape(1, C), (128, C))),
    }


def kernel(x, W_qkv, b_qkv, lora_kA, lora_kB, lora_vA, lora_vB, W_proj, b_proj):
    nc = build_nc(loop_reps=1)
    shared = _prep_shared(
        W_qkv, b_qkv, lora_kA, lora_kB, lora_vA, lora_vB, W_proj, b_proj
    )
    x = np.asarray(x, np.float32)
    in_maps = []
    for c in range(NCORES):
        xs = x[c * BSH : (c + 1) * BSH]
        xt = (
            np.ascontiguousarray(xs.transpose(0, 2, 1))
            .astype(BF_NP)
            .reshape(BSH, NB, 128, N)
        )
        in_maps.append({"xt": xt, **shared})
    res = run_bass_kernel_spmd(nc, in_maps, list(range(NCORES)))
    return np.concatenate(
        [res.results[c]["out"].astype(np.float32) for c in range(NCORES)], axis=0
    )
